# revision 18
# baseline (speedup 1.0000x reference)
"""MoE layer (straight-through, gate-token routing) on 8 trn2 NeuronCores.

Strategy:
  Launch 1 (gating, data-parallel): each core takes 512 tokens (x^T shard)
    and computes logits -> softmax -> argmax mask on device. Outputs per
    core: per-token expert ids, per-(token-tile, expert) prob sums and
    counts (partition-reduced on the tensor engine via a ones matmul).
  Host: shards tokens by expert id ("all-to-all" in host numpy), 2 cores
    per expert, fixed capacity C (padded with zero columns).
  Launch 2 (experts, expert-parallel, F-split): core 2e+h holds half of
    expert e's FFN (F/2 = 1536 columns of W1, matching rows of W2) and
    processes all of expert e's tokens:
        outT_part = W2h^T @ gelu(W1h^T @ xT + b1h)   (+ b2 on h==0 core)
    Matmuls run in fp32r (full-rate fp32 mode; inputs rounded on-chip).
    The two partial outputs of a pair are summed on the host (unshard of
    the F-split) and scattered back to token positions.
  balance_loss is computed on device (launch 2) from the globally summed
  gating partials, so every output value is device-computed.
"""

import sys

if "/opt/trn_rl_repo" not in sys.path:
    sys.path.insert(0, "/opt/trn_rl_repo")

import numpy as np

import concourse.bass as bass
import concourse.mybir as mybir
import concourse.tile as tile
from concourse import bacc
from concourse.bass_utils import run_bass_kernel_spmd

F32 = mybir.dt.float32
F32R = mybir.dt.float32r
AF = mybir.ActivationFunctionType
ALU = mybir.AluOpType
AX = mybir.AxisListType

B, S, D = 8, 512, 768
E, F = 4, 3072
N = B * S                 # 4096 tokens
TPC = N // 8              # 512 tokens per core in the gating launch
DC = D // 128             # 6 contraction chunks of 128
FH = F // 2               # 1536 F-columns per core (F-split across the pair)
FT = FH // 128            # 12 f-tiles per core
C_DEFAULT = 1152          # per-expert token capacity (mean load is 1024)

_CACHE = {}


class _nullcm:
    def __enter__(self):
        return self

    def __exit__(self, *a):
        return False


def _chunks(c):
    """Split capacity C into matmul moving-dim chunks, each in [256, 512]
    (fp32r runs full-rate only for free dim >= 256)."""
    out = []
    rem = c
    while rem >= 1024:
        out.append(512)
        rem -= 512
    if rem == 512:
        out.append(512)
    elif rem > 512:
        h1 = min(512, rem - 256)
        out.append(h1)
        out.append(rem - h1)
    elif rem > 0:
        out.append(rem)
    spans, t0 = [], 0
    for cn in out:
        spans.append((t0, cn))
        t0 += cn
    assert t0 == c
    return spans


def _build_gate():
    nc = bacc.Bacc("TRN2", target_bir_lowering=False, debug=False, num_devices=8)
    xT = nc.dram_tensor("xT", [D, TPC], F32, kind="ExternalInput")
    gwT = nc.dram_tensor("gwT", [128, DC, E], F32, kind="ExternalInput")
    iota = nc.dram_tensor("iota", [128, E], F32, kind="ExternalInput")
    NT = TPC // 128
    ids = nc.dram_tensor("ids", [128, NT], F32, kind="ExternalOutput")
    pc = nc.dram_tensor("pc", [2 * NT * E, 1], F32, kind="ExternalOutput")

    xr = xT.ap().rearrange("(c p) t -> c p t", p=128)

    with tile.TileContext(nc) as tc:
        with (
            tc.tile_pool(name="sb", bufs=2) as sb,
            tc.tile_pool(name="ps", bufs=3, space="PSUM") as ps,
            tc.tile_pool(name="psT", bufs=2, space="PSUM") as psT,
            tc.tile_pool(name="psc", bufs=1, space="PSUM") as psc,
        ):
            gw = sb.tile([128, DC, E], F32, tag="gw")
            with tc.high_priority():
                nc.sync.dma_start(gw[:], gwT.ap())
            xs = [
                sb.tile([128, TPC], F32, tag=f"x{dcI}", name=f"xs{dcI}")
                for dcI in range(DC)
            ]
            # sub-tile DMAs so matmuls for token-tile tt start as soon as
            # its 128 columns are in (not the whole 512)
            for tt in range(NT):
                for dcI in range(DC):
                    nc.sync.dma_start(
                        xs[dcI][:, tt * 128:(tt + 1) * 128],
                        xr[dcI][:, tt * 128:(tt + 1) * 128],
                    )
            io = sb.tile([128, E], F32, tag="io")
            nc.sync.dma_start(io[:], iota[:])
            ones = sb.tile([128, 1], F32, tag="ones")
            nc.vector.memset(ones[:], 1.0)
            ident = sb.tile([E, E], F32, tag="ident")
            from concourse.masks import make_identity
            make_identity(nc, ident[:])

            # logits^T per 128-token tile with gw as the 4-column stationary
            # (LDWEIGHTS ~4 cols instead of a 128-col fp32 x-tile), then a
            # tiny PE transpose back to [128 tokens, E]
            lg = sb.tile([128, NT, E], F32, tag="lg")
            for tt in range(NT):
                plT = psT.tile([E, 128], F32, tag="plT")
                for dcI in range(DC):
                    nc.tensor.matmul(
                        plT[:],
                        gw[:, dcI, :],
                        xs[dcI][:, tt * 128:(tt + 1) * 128],
                        start=(dcI == 0),
                        stop=(dcI == DC - 1),
                    )
                lgT = sb.tile([E, 128], F32, tag="lgT")
                nc.vector.tensor_copy(lgT[:], plT[:])
                pl = ps.tile([128, E], F32, tag="pl")
                nc.tensor.transpose(pl[:], lgT[:], ident[:])
                nc.vector.tensor_copy(lg[:, tt, :], pl[:])

            m4 = sb.tile([128, NT], F32, tag="m4")
            nc.vector.tensor_reduce(m4[:], lg[:], AX.X, ALU.max)
            mb = m4[:, :, None].broadcast_to([128, NT, E])
            # big: [probs (NT,E) | mask (NT,E)]
            big = sb.tile([128, 2 * NT * E], F32, tag="big")
            bigv = big[:].rearrange("p (k t e) -> p k t e", k=2, t=NT)
            ex = sb.tile([128, NT, E], F32, tag="ex")
            nc.vector.tensor_tensor(ex[:], lg[:], mb, ALU.subtract)
            nc.scalar.activation(ex[:], ex[:], AF.Exp)
            s4 = sb.tile([128, NT], F32, tag="s4")
            nc.vector.tensor_reduce(s4[:], ex[:], AX.X, ALU.add)
            r4 = sb.tile([128, NT], F32, tag="r4")
            nc.vector.reciprocal(r4[:], s4[:])
            rb = r4[:, :, None].broadcast_to([128, NT, E])
            nc.vector.tensor_tensor(bigv[:, 0], ex[:], rb, ALU.mult)
            nc.vector.tensor_tensor(bigv[:, 1], lg[:], mb, ALU.is_ge)
            # ids = sum_e e * mask
            iob = io[:, None, :].broadcast_to([128, NT, E])
            tmp = sb.tile([128, NT, E], F32, tag="tmp")
            nc.vector.tensor_tensor(tmp[:], bigv[:, 1], iob, ALU.mult)
            ids_sb = sb.tile([128, NT], F32, tag="ids")
            nc.vector.tensor_reduce(ids_sb[:], tmp[:], AX.X, ALU.add)
            # column sums over the 128 tokens on partitions: ones matmul
            ppc = psc.tile([2 * NT * E, 1], F32, tag="ppc")
            nc.tensor.matmul(ppc[:], big[:], ones[:], start=True, stop=True)
            pc_sb = sb.tile([2 * NT * E, 1], F32, tag="pc")
            nc.vector.tensor_copy(pc_sb[:], ppc[:])
            nc.sync.dma_start(pc[:], pc_sb[:])
            nc.sync.dma_start(ids[:], ids_sb[:])
    nc.compile()
    return nc


def _build_expert(cap):
    spans = _chunks(cap)
    NS = len(spans)
    nc = bacc.Bacc("TRN2", target_bir_lowering=False, debug=False, num_devices=8)
    xT = nc.dram_tensor("xT", [D, cap], F32, kind="ExternalInput")
    # w1 host layout: [FT, 128(p=f%128), DC, 128(d%128)] -> per-ft slab
    w1 = nc.dram_tensor("w1", [FT, 128, DC, 128], F32, kind="ExternalInput")
    b1 = nc.dram_tensor("b1", [128, FT], F32, kind="ExternalInput")
    w2 = nc.dram_tensor("w2", [FH, D], F32, kind="ExternalInput")
    b2 = nc.dram_tensor("b2", [128, DC], F32, kind="ExternalInput")
    pcin = nc.dram_tensor("pcin", [1, 2 * E], F32, kind="ExternalInput")
    outT = nc.dram_tensor("outT", [D, cap], F32, kind="ExternalOutput")
    bal = nc.dram_tensor("bal", [1, 1], F32, kind="ExternalOutput")

    xr = xT.ap().rearrange("(c p) t -> c p t", p=128)
    w2r_d = w2.ap().rearrange("(c p) d -> c p d", p=128)
    outr = outT.ap().rearrange("(c p) t -> c p t", p=128)

    with tile.TileContext(nc) as tc:
        with (
            tc.tile_pool(name="stage", bufs=3) as stage,
            tc.tile_pool(name="res", bufs=1) as res,
            tc.tile_pool(name="hp", bufs=1) as hp,
            tc.tile_pool(name="op", bufs=3) as op,
            tc.tile_pool(name="psA", bufs=4, space="PSUM") as psA,
            tc.tile_pool(name="psB", bufs=4, space="PSUM") as psB,
        ):
            b1_sb = res.tile([128, FT], F32, tag="b1")
            nc.sync.dma_start(b1_sb[:], b1.ap())
            b2_sb = res.tile([128, DC], F32, tag="b2")
            nc.sync.dma_start(b2_sb[:], b2.ap())

            # x: stream + round per (dc, span); w1 slabs interleaved so the
            # first matmuls (ft0, span0) have their inputs as early as
            # possible while later slabs stream during compute
            xrr = [[None] * NS for _ in range(DC)]
            w1t = [None] * FT

            def _load_w1(ft):
                st = stage.tile([128, DC, 128], F32, tag="w1s")
                nc.sync.dma_start(st[:], w1.ap()[ft])
                rt = res.tile([128, DC, 128], F32R, tag=f"w1r{ft}")
                nc.vector.tensor_copy(rt[:], st[:])
                w1t[ft] = rt

            # PE consumes: (ft0, span0), (ft0, span1), ... so emit x spans
            # first (all needed within the first ~10us) with one w1 slab
            # interleaved per span, then stream the remaining slabs
            for si, (t0, tn) in enumerate(spans):
                for dcI in range(DC):
                    st = stage.tile([128, tn], F32, tag="xs")
                    nc.sync.dma_start(st[:], xr[dcI][:, t0:t0 + tn])
                    rt = res.tile([128, tn], F32R, tag=f"xr{dcI}_{si}")
                    nc.vector.tensor_copy(rt[:], st[:])
                    xrr[dcI][si] = rt
                _load_w1(si)
            for ft in range(FT):
                if w1t[ft] is None:
                    _load_w1(ft)

            hs = []
            for ft in range(FT):
                h = hp.tile([128, cap], F32R, tag=f"h{ft}")
                hs.append(h)

            # layer 1: h = gelu(W1h^T @ xT + b1h), written as fp32r by ACT
            for ft in range(FT):
                for si, (t0, tn) in enumerate(spans):
                    ph = psA.tile([128, tn], F32, tag="ph")
                    for dcI in range(DC):
                        nc.tensor.matmul(
                            ph[:],
                            w1t[ft][:, dcI, :],
                            xrr[dcI][si][:],
                            start=(dcI == 0),
                            stop=(dcI == DC - 1),
                        )
                    nc.scalar.activation(
                        hs[ft][:, t0:t0 + tn], ph[:], AF.Gelu,
                        bias=b1_sb[:, ft:ft + 1], scale=1.0,
                    )

            # w2 loads are emitted after L1 so they don't crowd the early
            # DMA/DVE critical path; they complete during L1 compute
            w2t = []
            for fc in range(FT):
                st = stage.tile([128, D], F32, tag="w2s")
                nc.sync.dma_start(st[:], w2r_d[fc])
                rt = res.tile([128, D], F32R, tag=f"w2r{fc}")
                nc.vector.tensor_copy(rt[:], st[:])
                w2t.append(rt)

            # layer 2: outT = W2h^T @ h + b2 (d-major, per-partition bias)
            for si, (t0, tn) in enumerate(spans):
                for db in range(DC):
                    po = psB.tile([128, tn], F32, tag="po")
                    for fc in range(FT):
                        nc.tensor.matmul(
                            po[:],
                            w2t[fc][:, db * 128:(db + 1) * 128],
                            hs[fc][:, t0:t0 + tn],
                            start=(fc == 0),
                            stop=(fc == FT - 1),
                        )
                    ot = op.tile([128, tn], F32, tag="ot")
                    nc.scalar.activation(
                        ot[:], po[:], AF.Identity, bias=b2_sb[:, db:db + 1], scale=1.0,
                    )
                    nc.sync.dma_start(outr[db][:, t0:t0 + tn], ot[:])

            # balance loss from global gating partials
            pc_sb = res.tile([1, 2 * E], F32, tag="pc")
            nc.sync.dma_start(pc_sb[:], pcin.ap())
            tmp4 = res.tile([1, E], F32, tag="tmp4")
            nc.vector.tensor_tensor(tmp4[:], pc_sb[0:1, 0:E], pc_sb[0:1, E:2 * E], ALU.mult)
            bsum = res.tile([1, 1], F32, tag="bsum")
            nc.vector.tensor_reduce(bsum[:], tmp4[:], AX.X, ALU.add)
            bal_sb = res.tile([1, 1], F32, tag="bal")
            nc.scalar.mul(bal_sb[:], bsum[:], float(E) / float(N) / float(N))
            nc.sync.dma_start(bal.ap(), bal_sb[:])
    nc.compile()
    return nc


def _get_gate():
    if "gate" not in _CACHE:
        _CACHE["gate"] = _build_gate()
    return _CACHE["gate"]


def _get_expert(cap):
    key = ("exp", cap)
    if key not in _CACHE:
        _CACHE[key] = _build_expert(cap)
    return _CACHE[key]


def _run(inputs, trace=False):
    x = np.ascontiguousarray(np.asarray(inputs["x"], dtype=np.float32))
    gate_w = np.asarray(inputs["gate_w"], dtype=np.float32)
    W1 = np.asarray(inputs["W1"], dtype=np.float32)
    b1 = np.asarray(inputs["b1"], dtype=np.float32)
    W2 = np.asarray(inputs["W2"], dtype=np.float32)
    b2 = np.asarray(inputs["b2"], dtype=np.float32)

    xf = x.reshape(N, D)
    xT = np.ascontiguousarray(xf.T)                      # [768, 4096]
    # [128(p=d%128), DC, E]: contiguous per-partition lines for DMA
    gwT = np.ascontiguousarray(gate_w.T.reshape(DC, 128, E).transpose(1, 0, 2))
    iota = np.broadcast_to(
        np.arange(E, dtype=np.float32), (128, E)
    ).copy()

    # ---- launch 1: gating ----
    nc_g = _get_gate()
    in_maps = []
    for c in range(8):
        in_maps.append({
            "xT": np.ascontiguousarray(xT[:, c * TPC:(c + 1) * TPC]),
            "gwT": gwT,
            "iota": iota,
        })
    res_g = run_bass_kernel_spmd(nc_g, in_maps, core_ids=list(range(8)), trace=trace)
    t_gate = res_g.exec_time_ns

    NT = TPC // 128
    gate = np.concatenate(
        [res_g.results[c]["ids"].T.reshape(-1) for c in range(8)]
    )
    gate = np.rint(gate).astype(np.int64)
    # pc rows: [k(probs/mask), tt, e]
    pcs = np.sum([res_g.results[c]["pc"][:, 0] for c in range(8)], axis=0)
    pcs = pcs.reshape(2, NT, E).sum(axis=1)
    probsum, counts = pcs[0], pcs[1]
    gate_load = np.rint(counts).astype(np.int32)

    # ---- host all-to-all by gate id ----
    idx = [np.flatnonzero(gate == e) for e in range(E)]
    max_load = max(len(i) for i in idx)
    # cap > ~1400 would exceed SBUF; extreme imbalance runs multiple batches
    CAP_MAX = 1408
    if max_load <= CAP_MAX:
        cap = max(512, ((max_load + 31) // 32) * 32)
        n_batches = 1
    else:
        cap = CAP_MAX
        n_batches = -(-max_load // CAP_MAX)
    nc_e = _get_expert(cap)

    pcin = np.concatenate([probsum, counts]).astype(np.float32)[None, :]
    zeros_b2 = np.zeros_like(b2[0])
    wmaps = []
    for core in range(8):
        e, h = core // 2, core % 2
        w1h = W1[e][:, h * FH:(h + 1) * FH]              # [768, 1536]
        # -> [FT, 128(d%128), DC, 128(f%128)]: lhsT needs d on partitions
        w1h = np.ascontiguousarray(
            w1h.reshape(DC, 128, FT, 128).transpose(2, 1, 0, 3)
        )
        b1h = np.ascontiguousarray(b1[e][h * FH:(h + 1) * FH].reshape(FT, 128).T)
        w2h = np.ascontiguousarray(W2[e][h * FH:(h + 1) * FH, :])
        b2h = b2[e] if h == 0 else zeros_b2
        b2h = np.ascontiguousarray(b2h.reshape(DC, 128).T)
        wmaps.append({"w1": w1h, "b1": b1h, "w2": w2h, "b2": b2h, "pcin": pcin})

    out_flat = np.empty((N, D), np.float32)
    t_exp = 0
    balance_loss = np.float32(0.0)
    for bi in range(n_batches):
        bidx = [i[bi * cap:(bi + 1) * cap] for i in idx]
        in_maps = []
        xsel = []
        for e in range(E):
            xs = np.zeros((D, cap), np.float32)
            xs[:, :len(bidx[e])] = xT[:, bidx[e]]
            xsel.append(xs)
        for core in range(8):
            in_maps.append({"xT": xsel[core // 2], **wmaps[core]})
        res_e = run_bass_kernel_spmd(
            nc_e, in_maps, core_ids=list(range(8)), trace=trace
        )
        if res_e.exec_time_ns is not None:
            t_exp += res_e.exec_time_ns
        for e in range(E):
            if len(bidx[e]) == 0:
                continue
            oT = res_e.results[2 * e]["outT"] + res_e.results[2 * e + 1]["outT"]
            out_flat[bidx[e]] = oT[:, :len(bidx[e])].T
        balance_loss = np.float32(res_e.results[0]["bal"][0, 0])
    out = out_flat.reshape(B, S, D)
    if trace and t_exp == 0:
        t_exp = None

    times = (t_gate, t_exp)
    return (out, balance_loss, gate_load), times


def kernel(**inputs):
    (out, balance_loss, gate_load), _ = _run(inputs, trace=False)
    return out, balance_loss, gate_load


# revision 23
# speedup vs baseline: 1.0758x; 1.0758x over previous
"""MoE layer (straight-through, gate-token routing) on 8 trn2 NeuronCores.

Strategy:
  Launch 1 (gating, data-parallel): each core takes 512 tokens (x^T shard)
    and computes logits -> softmax -> argmax mask on device. Outputs per
    core: per-token expert ids, per-(token-tile, expert) prob sums and
    counts (partition-reduced on the tensor engine via a ones matmul).
  Host: shards tokens by expert id ("all-to-all" in host numpy), 2 cores
    per expert, fixed capacity C (padded with zero columns).
  Launch 2 (experts, expert-parallel, F-split): core 2e+h holds half of
    expert e's FFN (F/2 = 1536 columns of W1, matching rows of W2) and
    processes all of expert e's tokens:
        outT_part = W2h^T @ gelu(W1h^T @ xT + b1h)   (+ b2 on h==0 core)
    Matmuls run in fp32r (full-rate fp32 mode; inputs rounded on-chip).
    The two partial outputs of a pair are summed on the host (unshard of
    the F-split) and scattered back to token positions.
  balance_loss is computed on device (launch 2) from the globally summed
  gating partials, so every output value is device-computed.
"""

import sys

if "/opt/trn_rl_repo" not in sys.path:
    sys.path.insert(0, "/opt/trn_rl_repo")

import numpy as np

import concourse.bass as bass
import concourse.mybir as mybir
import concourse.tile as tile
from concourse import bacc
from concourse.bass_utils import run_bass_kernel_spmd

F32 = mybir.dt.float32
F32R = mybir.dt.float32r
AF = mybir.ActivationFunctionType
ALU = mybir.AluOpType
AX = mybir.AxisListType

B, S, D = 8, 512, 768
E, F = 4, 3072
N = B * S                 # 4096 tokens
TPC = N // 8              # 512 tokens per core in the gating launch
DC = D // 128             # 6 contraction chunks of 128
FH = F // 2               # 1536 F-columns per core (F-split across the pair)
FT = FH // 128            # 12 f-tiles per core
C_DEFAULT = 1152          # per-expert token capacity (mean load is 1024)

_CACHE = {}


class _nullcm:
    def __enter__(self):
        return self

    def __exit__(self, *a):
        return False


def _chunks(c):
    """Split capacity C into matmul moving-dim chunks, each in [256, 512]
    (fp32r runs full-rate only for free dim >= 256). Per-span cost is
    max(LDW ~169ns, N/2.4 ns) per matmul group, so spans <= ~406 cost a
    flat LDW-bound price: pick the span count k minimizing total cost
    with a balanced split."""
    if c <= 512:
        parts = [c]
    else:
        best, parts = None, None
        kmin = -(-c // 512)
        for k in range(kmin, kmin + 3):
            # balanced split in multiples of 4 (fp32r ISA restriction)
            q = c // 4
            base, ext = divmod(q, k)
            cand = [4 * (base + 1)] * ext + [4 * base] * (k - ext)
            if min(cand) < 256 or max(cand) > 512:
                continue
            cost = sum(max(169.0, n / 2.4) for n in cand)
            if best is None or cost < best:
                best, parts = cost, cand
        assert parts is not None
    spans, t0 = [], 0
    for cn in parts:
        spans.append((t0, cn))
        t0 += cn
    assert t0 == c
    return spans


def _build_gate():
    nc = bacc.Bacc("TRN2", target_bir_lowering=False, debug=False, num_devices=8)
    xT = nc.dram_tensor("xT", [D, TPC], F32, kind="ExternalInput")
    gwT = nc.dram_tensor("gwT", [128, DC, E], F32, kind="ExternalInput")
    iota = nc.dram_tensor("iota", [128, E], F32, kind="ExternalInput")
    NT = TPC // 128
    ids = nc.dram_tensor("ids", [128, NT], F32, kind="ExternalOutput")
    pc = nc.dram_tensor("pc", [2 * NT * E, 1], F32, kind="ExternalOutput")

    xr = xT.ap().rearrange("(c p) t -> c p t", p=128)

    with tile.TileContext(nc) as tc:
        with (
            tc.tile_pool(name="sb", bufs=2) as sb,
            tc.tile_pool(name="ps", bufs=3, space="PSUM") as ps,
            tc.tile_pool(name="psT", bufs=2, space="PSUM") as psT,
            tc.tile_pool(name="psc", bufs=1, space="PSUM") as psc,
        ):
            gw = sb.tile([128, DC, E], F32, tag="gw")
            with tc.high_priority():
                nc.sync.dma_start(gw[:], gwT.ap())
            xs = [
                sb.tile([128, TPC], F32, tag=f"x{dcI}", name=f"xs{dcI}")
                for dcI in range(DC)
            ]
            # sub-tile DMAs so matmuls for token-tile tt start as soon as
            # its 128 columns are in (not the whole 512)
            for tt in range(NT):
                for dcI in range(DC):
                    nc.sync.dma_start(
                        xs[dcI][:, tt * 128:(tt + 1) * 128],
                        xr[dcI][:, tt * 128:(tt + 1) * 128],
                    )
            io = sb.tile([128, E], F32, tag="io")
            nc.sync.dma_start(io[:], iota[:])
            ones = sb.tile([128, 1], F32, tag="ones")
            nc.vector.memset(ones[:], 1.0)
            ident = sb.tile([E, E], F32, tag="ident")
            from concourse.masks import make_identity
            make_identity(nc, ident[:])

            # logits^T per 128-token tile with gw as the 4-column stationary
            # (LDWEIGHTS ~4 cols instead of a 128-col fp32 x-tile), then a
            # tiny PE transpose back to [128 tokens, E]
            lg = sb.tile([128, NT, E], F32, tag="lg")
            for tt in range(NT):
                plT = psT.tile([E, 128], F32, tag="plT")
                for dcI in range(DC):
                    nc.tensor.matmul(
                        plT[:],
                        gw[:, dcI, :],
                        xs[dcI][:, tt * 128:(tt + 1) * 128],
                        start=(dcI == 0),
                        stop=(dcI == DC - 1),
                    )
                lgT = sb.tile([E, 128], F32, tag="lgT")
                nc.vector.tensor_copy(lgT[:], plT[:])
                pl = ps.tile([128, E], F32, tag="pl")
                nc.tensor.transpose(pl[:], lgT[:], ident[:])
                nc.vector.tensor_copy(lg[:, tt, :], pl[:])

            m4 = sb.tile([128, NT], F32, tag="m4")
            nc.vector.tensor_reduce(m4[:], lg[:], AX.X, ALU.max)
            mb = m4[:, :, None].broadcast_to([128, NT, E])
            # big: [probs (NT,E) | mask (NT,E)]
            big = sb.tile([128, 2 * NT * E], F32, tag="big")
            bigv = big[:].rearrange("p (k t e) -> p k t e", k=2, t=NT)
            ex = sb.tile([128, NT, E], F32, tag="ex")
            nc.vector.tensor_tensor(ex[:], lg[:], mb, ALU.subtract)
            nc.scalar.activation(ex[:], ex[:], AF.Exp)
            s4 = sb.tile([128, NT], F32, tag="s4")
            nc.vector.tensor_reduce(s4[:], ex[:], AX.X, ALU.add)
            r4 = sb.tile([128, NT], F32, tag="r4")
            nc.vector.reciprocal(r4[:], s4[:])
            rb = r4[:, :, None].broadcast_to([128, NT, E])
            nc.vector.tensor_tensor(bigv[:, 0], ex[:], rb, ALU.mult)
            nc.vector.tensor_tensor(bigv[:, 1], lg[:], mb, ALU.is_ge)
            # ids = sum_e e * mask
            iob = io[:, None, :].broadcast_to([128, NT, E])
            tmp = sb.tile([128, NT, E], F32, tag="tmp")
            nc.vector.tensor_tensor(tmp[:], bigv[:, 1], iob, ALU.mult)
            ids_sb = sb.tile([128, NT], F32, tag="ids")
            nc.vector.tensor_reduce(ids_sb[:], tmp[:], AX.X, ALU.add)
            # column sums over the 128 tokens on partitions: ones matmul
            ppc = psc.tile([2 * NT * E, 1], F32, tag="ppc")
            nc.tensor.matmul(ppc[:], big[:], ones[:], start=True, stop=True)
            pc_sb = sb.tile([2 * NT * E, 1], F32, tag="pc")
            nc.vector.tensor_copy(pc_sb[:], ppc[:])
            nc.sync.dma_start(pc[:], pc_sb[:])
            nc.sync.dma_start(ids[:], ids_sb[:])
    nc.compile()
    return nc


def _build_expert(cap):
    spans = _chunks(cap)
    NS = len(spans)
    nc = bacc.Bacc("TRN2", target_bir_lowering=False, debug=False, num_devices=8)
    xT = nc.dram_tensor("xT", [D, cap], F32, kind="ExternalInput")
    # w1 host layout: [FT, 128(p=f%128), DC, 128(d%128)] -> per-ft slab
    w1 = nc.dram_tensor("w1", [FT, 128, DC, 128], F32, kind="ExternalInput")
    b1 = nc.dram_tensor("b1", [128, FT], F32, kind="ExternalInput")
    w2 = nc.dram_tensor("w2", [FH, D], F32, kind="ExternalInput")
    b2 = nc.dram_tensor("b2", [128, DC], F32, kind="ExternalInput")
    pcin = nc.dram_tensor("pcin", [1, 2 * E], F32, kind="ExternalInput")
    outT = nc.dram_tensor("outT", [D, cap], F32, kind="ExternalOutput")
    bal = nc.dram_tensor("bal", [1, 1], F32, kind="ExternalOutput")

    xr = xT.ap().rearrange("(c p) t -> c p t", p=128)
    w2r_d = w2.ap().rearrange("(c p) d -> c p d", p=128)
    outr = outT.ap().rearrange("(c p) t -> c p t", p=128)

    with tile.TileContext(nc) as tc:
        with (
            tc.tile_pool(name="stage", bufs=3) as stage,
            tc.tile_pool(name="res", bufs=1) as res,
            tc.tile_pool(name="hp", bufs=1) as hp,
            tc.tile_pool(name="op", bufs=3) as op,
            tc.tile_pool(name="psA", bufs=4, space="PSUM") as psA,
            tc.tile_pool(name="psB", bufs=4, space="PSUM") as psB,
        ):
            b1_sb = res.tile([128, FT], F32, tag="b1")
            nc.sync.dma_start(b1_sb[:], b1.ap())
            b2_sb = res.tile([128, DC], F32, tag="b2")
            nc.sync.dma_start(b2_sb[:], b2.ap())

            # x: stream + round per (dc, span); w1 slabs interleaved so the
            # first matmuls (ft0, span0) have their inputs as early as
            # possible while later slabs stream during compute
            xrr = [[None] * NS for _ in range(DC)]
            w1t = [None] * FT

            def _load_w1(ft):
                st = stage.tile([128, DC, 128], F32, tag="w1s")
                nc.sync.dma_start(st[:], w1.ap()[ft])
                rt = res.tile([128, DC, 128], F32R, tag=f"w1r{ft}")
                nc.vector.tensor_copy(rt[:], st[:])
                w1t[ft] = rt

            # PE consumes: (ft0, span0), (ft0, span1), ... so emit x spans
            # first (all needed within the first ~10us) with one w1 slab
            # interleaved per span, then stream the remaining slabs
            for si, (t0, tn) in enumerate(spans):
                for dcI in range(DC):
                    st = stage.tile([128, tn], F32, tag="xs")
                    nc.sync.dma_start(st[:], xr[dcI][:, t0:t0 + tn])
                    rt = res.tile([128, tn], F32R, tag=f"xr{dcI}_{si}")
                    nc.vector.tensor_copy(rt[:], st[:])
                    xrr[dcI][si] = rt
                _load_w1(si)
            for ft in range(FT):
                if w1t[ft] is None:
                    _load_w1(ft)

            hs = []
            for ft in range(FT):
                h = hp.tile([128, cap], F32R, tag=f"h{ft}")
                hs.append(h)

            # layer 1: h = gelu(W1h^T @ xT + b1h), written as fp32r by ACT
            for ft in range(FT):
                for si, (t0, tn) in enumerate(spans):
                    ph = psA.tile([128, tn], F32, tag="ph")
                    for dcI in range(DC):
                        nc.tensor.matmul(
                            ph[:],
                            w1t[ft][:, dcI, :],
                            xrr[dcI][si][:],
                            start=(dcI == 0),
                            stop=(dcI == DC - 1),
                        )
                    nc.scalar.activation(
                        hs[ft][:, t0:t0 + tn], ph[:], AF.Gelu,
                        bias=b1_sb[:, ft:ft + 1], scale=1.0,
                    )

            # w2 loads are emitted after L1 so they don't crowd the early
            # DMA/DVE critical path; they complete during L1 compute
            w2t = []
            for fc in range(FT):
                st = stage.tile([128, D], F32, tag="w2s")
                nc.sync.dma_start(st[:], w2r_d[fc])
                rt = res.tile([128, D], F32R, tag=f"w2r{fc}")
                nc.vector.tensor_copy(rt[:], st[:])
                w2t.append(rt)

            # layer 2: outT = W2h^T @ h + b2 (d-major, per-partition bias)
            for si, (t0, tn) in enumerate(spans):
                for db in range(DC):
                    po = psB.tile([128, tn], F32, tag="po")
                    for fc in range(FT):
                        nc.tensor.matmul(
                            po[:],
                            w2t[fc][:, db * 128:(db + 1) * 128],
                            hs[fc][:, t0:t0 + tn],
                            start=(fc == 0),
                            stop=(fc == FT - 1),
                        )
                    ot = op.tile([128, tn], F32, tag="ot")
                    nc.scalar.activation(
                        ot[:], po[:], AF.Identity, bias=b2_sb[:, db:db + 1], scale=1.0,
                    )
                    nc.sync.dma_start(outr[db][:, t0:t0 + tn], ot[:])

            # balance loss from global gating partials
            pc_sb = res.tile([1, 2 * E], F32, tag="pc")
            nc.sync.dma_start(pc_sb[:], pcin.ap())
            tmp4 = res.tile([1, E], F32, tag="tmp4")
            nc.vector.tensor_tensor(tmp4[:], pc_sb[0:1, 0:E], pc_sb[0:1, E:2 * E], ALU.mult)
            bsum = res.tile([1, 1], F32, tag="bsum")
            nc.vector.tensor_reduce(bsum[:], tmp4[:], AX.X, ALU.add)
            bal_sb = res.tile([1, 1], F32, tag="bal")
            nc.scalar.mul(bal_sb[:], bsum[:], float(E) / float(N) / float(N))
            nc.sync.dma_start(bal.ap(), bal_sb[:])
    nc.compile()
    return nc


def _get_gate():
    if "gate" not in _CACHE:
        _CACHE["gate"] = _build_gate()
    return _CACHE["gate"]


def _get_expert(cap):
    key = ("exp", cap)
    if key not in _CACHE:
        _CACHE[key] = _build_expert(cap)
    return _CACHE[key]


def _run(inputs, trace=False):
    x = np.ascontiguousarray(np.asarray(inputs["x"], dtype=np.float32))
    gate_w = np.asarray(inputs["gate_w"], dtype=np.float32)
    W1 = np.asarray(inputs["W1"], dtype=np.float32)
    b1 = np.asarray(inputs["b1"], dtype=np.float32)
    W2 = np.asarray(inputs["W2"], dtype=np.float32)
    b2 = np.asarray(inputs["b2"], dtype=np.float32)

    xf = x.reshape(N, D)
    xT = np.ascontiguousarray(xf.T)                      # [768, 4096]
    # [128(p=d%128), DC, E]: contiguous per-partition lines for DMA
    gwT = np.ascontiguousarray(gate_w.T.reshape(DC, 128, E).transpose(1, 0, 2))
    iota = np.broadcast_to(
        np.arange(E, dtype=np.float32), (128, E)
    ).copy()

    # ---- launch 1: gating ----
    nc_g = _get_gate()
    in_maps = []
    for c in range(8):
        in_maps.append({
            "xT": np.ascontiguousarray(xT[:, c * TPC:(c + 1) * TPC]),
            "gwT": gwT,
            "iota": iota,
        })
    res_g = run_bass_kernel_spmd(nc_g, in_maps, core_ids=list(range(8)), trace=trace)
    t_gate = res_g.exec_time_ns

    NT = TPC // 128
    gate = np.concatenate(
        [res_g.results[c]["ids"].T.reshape(-1) for c in range(8)]
    )
    gate = np.rint(gate).astype(np.int64)
    # pc rows: [k(probs/mask), tt, e]
    pcs = np.sum([res_g.results[c]["pc"][:, 0] for c in range(8)], axis=0)
    pcs = pcs.reshape(2, NT, E).sum(axis=1)
    probsum, counts = pcs[0], pcs[1]
    gate_load = np.rint(counts).astype(np.int32)

    # ---- host all-to-all by gate id ----
    idx = [np.flatnonzero(gate == e) for e in range(E)]
    max_load = max(len(i) for i in idx)
    # cap > ~1400 would exceed SBUF; extreme imbalance runs multiple batches
    CAP_MAX = 1408
    if max_load <= CAP_MAX:
        cap = max(512, ((max_load + 31) // 32) * 32)
        n_batches = 1
    else:
        cap = CAP_MAX
        n_batches = -(-max_load // CAP_MAX)
    nc_e = _get_expert(cap)

    pcin = np.concatenate([probsum, counts]).astype(np.float32)[None, :]
    zeros_b2 = np.zeros_like(b2[0])
    wmaps = []
    for core in range(8):
        e, h = core // 2, core % 2
        w1h = W1[e][:, h * FH:(h + 1) * FH]              # [768, 1536]
        # -> [FT, 128(d%128), DC, 128(f%128)]: lhsT needs d on partitions
        w1h = np.ascontiguousarray(
            w1h.reshape(DC, 128, FT, 128).transpose(2, 1, 0, 3)
        )
        b1h = np.ascontiguousarray(b1[e][h * FH:(h + 1) * FH].reshape(FT, 128).T)
        w2h = np.ascontiguousarray(W2[e][h * FH:(h + 1) * FH, :])
        b2h = b2[e] if h == 0 else zeros_b2
        b2h = np.ascontiguousarray(b2h.reshape(DC, 128).T)
        wmaps.append({"w1": w1h, "b1": b1h, "w2": w2h, "b2": b2h, "pcin": pcin})

    out_flat = np.empty((N, D), np.float32)
    t_exp = 0
    balance_loss = np.float32(0.0)
    for bi in range(n_batches):
        bidx = [i[bi * cap:(bi + 1) * cap] for i in idx]
        in_maps = []
        xsel = []
        for e in range(E):
            xs = np.zeros((D, cap), np.float32)
            xs[:, :len(bidx[e])] = xT[:, bidx[e]]
            xsel.append(xs)
        for core in range(8):
            in_maps.append({"xT": xsel[core // 2], **wmaps[core]})
        res_e = run_bass_kernel_spmd(
            nc_e, in_maps, core_ids=list(range(8)), trace=trace
        )
        if res_e.exec_time_ns is not None:
            t_exp += res_e.exec_time_ns
        for e in range(E):
            if len(bidx[e]) == 0:
                continue
            oT = res_e.results[2 * e]["outT"] + res_e.results[2 * e + 1]["outT"]
            out_flat[bidx[e]] = oT[:, :len(bidx[e])].T
        balance_loss = np.float32(res_e.results[0]["bal"][0, 0])
    out = out_flat.reshape(B, S, D)
    if trace and t_exp == 0:
        t_exp = None

    times = (t_gate, t_exp)
    return (out, balance_loss, gate_load), times


def kernel(**inputs):
    (out, balance_loss, gate_load), _ = _run(inputs, trace=False)
    return out, balance_loss, gate_load


# revision 26
# speedup vs baseline: 1.0886x; 1.0119x over previous
"""MoE layer (straight-through, gate-token routing) on 8 trn2 NeuronCores.

Strategy:
  Launch 1 (gating, data-parallel): each core takes 512 tokens (x^T shard)
    and computes logits -> softmax -> argmax mask on device. Outputs per
    core: per-token expert ids, per-(token-tile, expert) prob sums and
    counts (partition-reduced on the tensor engine via a ones matmul).
  Host: shards tokens by expert id ("all-to-all" in host numpy), 2 cores
    per expert, fixed capacity C (padded with zero columns).
  Launch 2 (experts, expert-parallel, F-split): core 2e+h holds half of
    expert e's FFN (F/2 = 1536 columns of W1, matching rows of W2) and
    processes all of expert e's tokens:
        outT_part = W2h^T @ gelu(W1h^T @ xT + b1h)   (+ b2 on h==0 core)
    Matmuls run in fp32r (full-rate fp32 mode; inputs rounded on-chip).
    The two partial outputs of a pair are summed on the host (unshard of
    the F-split) and scattered back to token positions.
  balance_loss is computed on device (launch 2) from the globally summed
  gating partials, so every output value is device-computed.
"""

import sys

if "/opt/trn_rl_repo" not in sys.path:
    sys.path.insert(0, "/opt/trn_rl_repo")

import numpy as np

import concourse.bass as bass
import concourse.mybir as mybir
import concourse.tile as tile
from concourse import bacc
from concourse.bass_utils import run_bass_kernel_spmd

F32 = mybir.dt.float32
F32R = mybir.dt.float32r
AF = mybir.ActivationFunctionType
ALU = mybir.AluOpType
AX = mybir.AxisListType

B, S, D = 8, 512, 768
E, F = 4, 3072
N = B * S                 # 4096 tokens
TPC = N // 8              # 512 tokens per core in the gating launch
DC = D // 128             # 6 contraction chunks of 128
FH = F // 2               # 1536 F-columns per core (F-split across the pair)
FT = FH // 128            # 12 f-tiles per core
C_DEFAULT = 1152          # per-expert token capacity (mean load is 1024)

_CACHE = {}


class _nullcm:
    def __enter__(self):
        return self

    def __exit__(self, *a):
        return False


def _chunks(c):
    """Split capacity C into matmul moving-dim chunks, each in [256, 512]
    (fp32r runs full-rate only for free dim >= 256). Per-span cost is
    max(LDW ~169ns, N/2.4 ns) per matmul group, so spans <= ~406 cost a
    flat LDW-bound price: pick the span count k minimizing total cost
    with a balanced split."""
    if c <= 512:
        parts = [c]
    else:
        best, parts = None, None
        kmin = -(-c // 512)
        for k in range(kmin, kmin + 3):
            # balanced split in multiples of 4 (fp32r ISA restriction)
            q = c // 4
            base, ext = divmod(q, k)
            cand = [4 * (base + 1)] * ext + [4 * base] * (k - ext)
            if min(cand) < 256 or max(cand) > 512:
                continue
            cost = sum(max(169.0, n / 2.4) for n in cand)
            if best is None or cost < best:
                best, parts = cost, cand
        assert parts is not None
    spans, t0 = [], 0
    for cn in parts:
        spans.append((t0, cn))
        t0 += cn
    assert t0 == c
    return spans


def _build_gate():
    nc = bacc.Bacc("TRN2", target_bir_lowering=False, debug=False, num_devices=8)
    xT = nc.dram_tensor("xT", [D, TPC], F32, kind="ExternalInput")
    gwT = nc.dram_tensor("gwT", [128, DC, E], F32, kind="ExternalInput")
    iota = nc.dram_tensor("iota", [128, E], F32, kind="ExternalInput")
    NT = TPC // 128
    ids = nc.dram_tensor("ids", [128, NT], F32, kind="ExternalOutput")
    pc = nc.dram_tensor("pc", [2 * NT * E, 1], F32, kind="ExternalOutput")

    xr = xT.ap().rearrange("(c p) t -> c p t", p=128)

    with tile.TileContext(nc) as tc:
        with (
            tc.tile_pool(name="sb", bufs=2) as sb,
            tc.tile_pool(name="ps", bufs=3, space="PSUM") as ps,
            tc.tile_pool(name="psT", bufs=2, space="PSUM") as psT,
            tc.tile_pool(name="psc", bufs=1, space="PSUM") as psc,
        ):
            gw = sb.tile([128, DC, E], F32, tag="gw")
            with tc.high_priority():
                nc.sync.dma_start(gw[:], gwT.ap())
            xs = [
                sb.tile([128, TPC], F32, tag=f"x{dcI}", name=f"xs{dcI}")
                for dcI in range(DC)
            ]
            # sub-tile DMAs so matmuls for token-tile tt start as soon as
            # its 128 columns are in (not the whole 512)
            for tt in range(NT):
                for dcI in range(DC):
                    nc.sync.dma_start(
                        xs[dcI][:, tt * 128:(tt + 1) * 128],
                        xr[dcI][:, tt * 128:(tt + 1) * 128],
                    )
            io = sb.tile([128, E], F32, tag="io")
            nc.sync.dma_start(io[:], iota[:])
            ones = sb.tile([128, 1], F32, tag="ones")
            nc.vector.memset(ones[:], 1.0)
            ident = sb.tile([E, E], F32, tag="ident")
            from concourse.masks import make_identity
            make_identity(nc, ident[:])

            # logits^T per 128-token tile with gw as the 4-column stationary
            # (LDWEIGHTS ~4 cols instead of a 128-col fp32 x-tile), then a
            # tiny PE transpose back to [128 tokens, E]
            lg = sb.tile([128, NT, E], F32, tag="lg")
            for tt in range(NT):
                plT = psT.tile([E, 128], F32, tag="plT")
                for dcI in range(DC):
                    nc.tensor.matmul(
                        plT[:],
                        gw[:, dcI, :],
                        xs[dcI][:, tt * 128:(tt + 1) * 128],
                        start=(dcI == 0),
                        stop=(dcI == DC - 1),
                    )
                lgT = sb.tile([E, 128], F32, tag="lgT")
                nc.vector.tensor_copy(lgT[:], plT[:])
                pl = ps.tile([128, E], F32, tag="pl")
                nc.tensor.transpose(pl[:], lgT[:], ident[:])
                nc.vector.tensor_copy(lg[:, tt, :], pl[:])

            m4 = sb.tile([128, NT], F32, tag="m4")
            nc.vector.tensor_reduce(m4[:], lg[:], AX.X, ALU.max)
            mb = m4[:, :, None].broadcast_to([128, NT, E])
            # big: [probs (NT,E) | mask (NT,E)]
            big = sb.tile([128, 2 * NT * E], F32, tag="big")
            bigv = big[:].rearrange("p (k t e) -> p k t e", k=2, t=NT)
            ex = sb.tile([128, NT, E], F32, tag="ex")
            nc.vector.tensor_tensor(ex[:], lg[:], mb, ALU.subtract)
            nc.scalar.activation(ex[:], ex[:], AF.Exp)
            s4 = sb.tile([128, NT], F32, tag="s4")
            nc.vector.tensor_reduce(s4[:], ex[:], AX.X, ALU.add)
            r4 = sb.tile([128, NT], F32, tag="r4")
            nc.vector.reciprocal(r4[:], s4[:])
            rb = r4[:, :, None].broadcast_to([128, NT, E])
            nc.vector.tensor_tensor(bigv[:, 0], ex[:], rb, ALU.mult)
            nc.vector.tensor_tensor(bigv[:, 1], lg[:], mb, ALU.is_ge)
            # ids = sum_e e * mask
            iob = io[:, None, :].broadcast_to([128, NT, E])
            tmp = sb.tile([128, NT, E], F32, tag="tmp")
            nc.vector.tensor_tensor(tmp[:], bigv[:, 1], iob, ALU.mult)
            ids_sb = sb.tile([128, NT], F32, tag="ids")
            nc.vector.tensor_reduce(ids_sb[:], tmp[:], AX.X, ALU.add)
            # column sums over the 128 tokens on partitions: ones matmul
            ppc = psc.tile([2 * NT * E, 1], F32, tag="ppc")
            nc.tensor.matmul(ppc[:], big[:], ones[:], start=True, stop=True)
            pc_sb = sb.tile([2 * NT * E, 1], F32, tag="pc")
            nc.vector.tensor_copy(pc_sb[:], ppc[:])
            nc.sync.dma_start(pc[:], pc_sb[:])
            nc.sync.dma_start(ids[:], ids_sb[:])
    nc.compile()
    return nc


def _build_expert(cap):
    # first span is a host-packed 256-column "first bite" (single DMA with
    # 6KB contiguous lines) so the first matmul group starts ~5us earlier
    if cap > 512:
        spans = [(0, 256)] + [(t0 + 256, tn) for t0, tn in _chunks(cap - 256)]
    else:
        spans = _chunks(cap)
    NS = len(spans)
    nc = bacc.Bacc("TRN2", target_bir_lowering=False, debug=False, num_devices=8)
    xT = nc.dram_tensor("xT", [D, cap], F32, kind="ExternalInput")
    x0 = nc.dram_tensor("x0", [128, DC, 256], F32, kind="ExternalInput")
    # w1 host layout: [FT, 128(p=f%128), DC, 128(d%128)] -> per-ft slab
    w1 = nc.dram_tensor("w1", [FT, 128, DC, 128], F32, kind="ExternalInput")
    b1 = nc.dram_tensor("b1", [128, FT], F32, kind="ExternalInput")
    w2 = nc.dram_tensor("w2", [FH, D], F32, kind="ExternalInput")
    b2 = nc.dram_tensor("b2", [128, DC], F32, kind="ExternalInput")
    pcin = nc.dram_tensor("pcin", [1, 2 * E], F32, kind="ExternalInput")
    outT = nc.dram_tensor("outT", [D, cap], F32, kind="ExternalOutput")
    bal = nc.dram_tensor("bal", [1, 1], F32, kind="ExternalOutput")

    xr = xT.ap().rearrange("(c p) t -> c p t", p=128)
    w2r_d = w2.ap().rearrange("(c p) d -> c p d", p=128)
    outr = outT.ap().rearrange("(c p) t -> c p t", p=128)

    with tile.TileContext(nc) as tc:
        with (
            tc.tile_pool(name="stage", bufs=3) as stage,
            tc.tile_pool(name="res", bufs=1) as res,
            tc.tile_pool(name="hp", bufs=1) as hp,
            tc.tile_pool(name="op", bufs=3) as op,
            tc.tile_pool(name="psA", bufs=4, space="PSUM") as psA,
            tc.tile_pool(name="psB", bufs=4, space="PSUM") as psB,
        ):
            b1_sb = res.tile([128, FT], F32, tag="b1")
            nc.sync.dma_start(b1_sb[:], b1.ap())
            b2_sb = res.tile([128, DC], F32, tag="b2")
            nc.sync.dma_start(b2_sb[:], b2.ap())

            # x: stream + round per (dc, span); w1 slabs interleaved so the
            # first matmuls (ft0, span0) have their inputs as early as
            # possible while later slabs stream during compute
            xrr = [[None] * NS for _ in range(DC)]
            w1t = [None] * FT

            def _load_w1(ft):
                st = stage.tile([128, DC, 128], F32, tag="w1s")
                nc.sync.dma_start(st[:], w1.ap()[ft])
                rt = res.tile([128, DC, 128], F32R, tag=f"w1r{ft}")
                nc.vector.tensor_copy(rt[:], st[:])
                w1t[ft] = rt

            # PE consumes: (ft0, span0), (ft0, span1), ... so emit x spans
            # first (all needed within the first ~10us) with one w1 slab
            # interleaved per span, then stream the remaining slabs
            fastbite = cap > 512
            if fastbite:
                x0st = stage.tile([128, DC, 256], F32, tag="x0s")
                nc.sync.dma_start(x0st[:], x0.ap())
                for dcI in range(DC):
                    rt = res.tile([128, 256], F32R, tag=f"xr{dcI}_0", name=f"x0r{dcI}")
                    nc.vector.tensor_copy(rt[:], x0st[:, dcI, :])
                    xrr[dcI][0] = rt
                _load_w1(0)
            for si, (t0, tn) in enumerate(spans):
                if fastbite and si == 0:
                    continue
                for dcI in range(DC):
                    st = stage.tile([128, tn], F32, tag="xs")
                    nc.sync.dma_start(st[:], xr[dcI][:, t0:t0 + tn])
                    rt = res.tile([128, tn], F32R, tag=f"xr{dcI}_{si}")
                    nc.vector.tensor_copy(rt[:], st[:])
                    xrr[dcI][si] = rt
                if si < FT:
                    _load_w1(si)
            for ft in range(FT):
                if w1t[ft] is None:
                    _load_w1(ft)

            hs = []
            for ft in range(FT):
                h = hp.tile([128, cap], F32R, tag=f"h{ft}")
                hs.append(h)

            # layer 1: h = gelu(W1h^T @ xT + b1h), written as fp32r by ACT
            for ft in range(FT):
                for si, (t0, tn) in enumerate(spans):
                    ph = psA.tile([128, tn], F32, tag="ph")
                    for dcI in range(DC):
                        nc.tensor.matmul(
                            ph[:],
                            w1t[ft][:, dcI, :],
                            xrr[dcI][si][:],
                            start=(dcI == 0),
                            stop=(dcI == DC - 1),
                        )
                    nc.scalar.activation(
                        hs[ft][:, t0:t0 + tn], ph[:], AF.Gelu,
                        bias=b1_sb[:, ft:ft + 1], scale=1.0,
                    )

            # w2 loads are emitted after L1 so they don't crowd the early
            # DMA/DVE critical path; they complete during L1 compute
            w2t = []
            for fc in range(FT):
                st = stage.tile([128, D], F32, tag="w2s")
                nc.sync.dma_start(st[:], w2r_d[fc])
                rt = res.tile([128, D], F32R, tag=f"w2r{fc}")
                nc.vector.tensor_copy(rt[:], st[:])
                w2t.append(rt)

            # layer 2: outT = W2h^T @ h + b2 (d-major, per-partition bias)
            for si, (t0, tn) in enumerate(spans):
                for db in range(DC):
                    po = psB.tile([128, tn], F32, tag="po")
                    for fc in range(FT):
                        nc.tensor.matmul(
                            po[:],
                            w2t[fc][:, db * 128:(db + 1) * 128],
                            hs[fc][:, t0:t0 + tn],
                            start=(fc == 0),
                            stop=(fc == FT - 1),
                        )
                    ot = op.tile([128, tn], F32, tag="ot")
                    nc.scalar.activation(
                        ot[:], po[:], AF.Identity, bias=b2_sb[:, db:db + 1], scale=1.0,
                    )
                    nc.sync.dma_start(outr[db][:, t0:t0 + tn], ot[:])

            # balance loss from global gating partials
            pc_sb = res.tile([1, 2 * E], F32, tag="pc")
            nc.sync.dma_start(pc_sb[:], pcin.ap())
            tmp4 = res.tile([1, E], F32, tag="tmp4")
            nc.vector.tensor_tensor(tmp4[:], pc_sb[0:1, 0:E], pc_sb[0:1, E:2 * E], ALU.mult)
            bsum = res.tile([1, 1], F32, tag="bsum")
            nc.vector.tensor_reduce(bsum[:], tmp4[:], AX.X, ALU.add)
            bal_sb = res.tile([1, 1], F32, tag="bal")
            nc.scalar.mul(bal_sb[:], bsum[:], float(E) / float(N) / float(N))
            nc.sync.dma_start(bal.ap(), bal_sb[:])
    nc.compile()
    return nc


def _get_gate():
    if "gate" not in _CACHE:
        _CACHE["gate"] = _build_gate()
    return _CACHE["gate"]


def _get_expert(cap):
    key = ("exp", cap)
    if key not in _CACHE:
        _CACHE[key] = _build_expert(cap)
    return _CACHE[key]


def _run(inputs, trace=False):
    x = np.ascontiguousarray(np.asarray(inputs["x"], dtype=np.float32))
    gate_w = np.asarray(inputs["gate_w"], dtype=np.float32)
    W1 = np.asarray(inputs["W1"], dtype=np.float32)
    b1 = np.asarray(inputs["b1"], dtype=np.float32)
    W2 = np.asarray(inputs["W2"], dtype=np.float32)
    b2 = np.asarray(inputs["b2"], dtype=np.float32)

    xf = x.reshape(N, D)
    xT = np.ascontiguousarray(xf.T)                      # [768, 4096]
    # [128(p=d%128), DC, E]: contiguous per-partition lines for DMA
    gwT = np.ascontiguousarray(gate_w.T.reshape(DC, 128, E).transpose(1, 0, 2))
    iota = np.broadcast_to(
        np.arange(E, dtype=np.float32), (128, E)
    ).copy()

    # ---- launch 1: gating ----
    nc_g = _get_gate()
    in_maps = []
    for c in range(8):
        in_maps.append({
            "xT": np.ascontiguousarray(xT[:, c * TPC:(c + 1) * TPC]),
            "gwT": gwT,
            "iota": iota,
        })
    res_g = run_bass_kernel_spmd(nc_g, in_maps, core_ids=list(range(8)), trace=trace)
    t_gate = res_g.exec_time_ns

    NT = TPC // 128
    gate = np.concatenate(
        [res_g.results[c]["ids"].T.reshape(-1) for c in range(8)]
    )
    gate = np.rint(gate).astype(np.int64)
    # pc rows: [k(probs/mask), tt, e]
    pcs = np.sum([res_g.results[c]["pc"][:, 0] for c in range(8)], axis=0)
    pcs = pcs.reshape(2, NT, E).sum(axis=1)
    probsum, counts = pcs[0], pcs[1]
    gate_load = np.rint(counts).astype(np.int32)

    # ---- host all-to-all by gate id ----
    idx = [np.flatnonzero(gate == e) for e in range(E)]
    max_load = max(len(i) for i in idx)
    # cap > ~1400 would exceed SBUF; extreme imbalance runs multiple batches
    CAP_MAX = 1408
    if max_load <= CAP_MAX:
        cap = max(512, ((max_load + 31) // 32) * 32)
        n_batches = 1
    else:
        cap = CAP_MAX
        n_batches = -(-max_load // CAP_MAX)
    nc_e = _get_expert(cap)

    pcin = np.concatenate([probsum, counts]).astype(np.float32)[None, :]
    zeros_b2 = np.zeros_like(b2[0])
    wmaps = []
    for core in range(8):
        e, h = core // 2, core % 2
        w1h = W1[e][:, h * FH:(h + 1) * FH]              # [768, 1536]
        # -> [FT, 128(d%128), DC, 128(f%128)]: lhsT needs d on partitions
        w1h = np.ascontiguousarray(
            w1h.reshape(DC, 128, FT, 128).transpose(2, 1, 0, 3)
        )
        b1h = np.ascontiguousarray(b1[e][h * FH:(h + 1) * FH].reshape(FT, 128).T)
        w2h = np.ascontiguousarray(W2[e][h * FH:(h + 1) * FH, :])
        b2h = b2[e] if h == 0 else zeros_b2
        b2h = np.ascontiguousarray(b2h.reshape(DC, 128).T)
        wmaps.append({"w1": w1h, "b1": b1h, "w2": w2h, "b2": b2h, "pcin": pcin})

    out_flat = np.empty((N, D), np.float32)
    t_exp = 0
    balance_loss = np.float32(0.0)
    for bi in range(n_batches):
        bidx = [i[bi * cap:(bi + 1) * cap] for i in idx]
        in_maps = []
        xsel = []
        x0sel = []
        for e in range(E):
            xs = np.zeros((D, cap), np.float32)
            xs[:, :len(bidx[e])] = xT[:, bidx[e]]
            xsel.append(xs)
            if cap > 512:
                x0 = np.ascontiguousarray(
                    xs[:, :256].reshape(DC, 128, 256).transpose(1, 0, 2)
                )
            else:
                x0 = np.zeros((128, DC, 256), np.float32)
            x0sel.append(x0)
        for core in range(8):
            in_maps.append(
                {"xT": xsel[core // 2], "x0": x0sel[core // 2], **wmaps[core]}
            )
        res_e = run_bass_kernel_spmd(
            nc_e, in_maps, core_ids=list(range(8)), trace=trace
        )
        if res_e.exec_time_ns is not None:
            t_exp += res_e.exec_time_ns
        for e in range(E):
            if len(bidx[e]) == 0:
                continue
            oT = res_e.results[2 * e]["outT"] + res_e.results[2 * e + 1]["outT"]
            out_flat[bidx[e]] = oT[:, :len(bidx[e])].T
        balance_loss = np.float32(res_e.results[0]["bal"][0, 0])
    out = out_flat.reshape(B, S, D)
    if trace and t_exp == 0:
        t_exp = None

    times = (t_gate, t_exp)
    return (out, balance_loss, gate_load), times


def kernel(**inputs):
    (out, balance_loss, gate_load), _ = _run(inputs, trace=False)
    return out, balance_loss, gate_load


# revision 28
# speedup vs baseline: 1.1061x; 1.0161x over previous
"""MoE layer (straight-through, gate-token routing) on 8 trn2 NeuronCores.

Strategy:
  Launch 1 (gating, data-parallel): each core takes 512 tokens (x^T shard)
    and computes logits -> softmax -> argmax mask on device. Outputs per
    core: per-token expert ids, per-(token-tile, expert) prob sums and
    counts (partition-reduced on the tensor engine via a ones matmul).
  Host: shards tokens by expert id ("all-to-all" in host numpy), 2 cores
    per expert, fixed capacity C (padded with zero columns).
  Launch 2 (experts, expert-parallel, F-split): core 2e+h holds half of
    expert e's FFN (F/2 = 1536 columns of W1, matching rows of W2) and
    processes all of expert e's tokens:
        outT_part = W2h^T @ gelu(W1h^T @ xT + b1h)   (+ b2 on h==0 core)
    Matmuls run in fp32r (full-rate fp32 mode; inputs rounded on-chip).
    The two partial outputs of a pair are summed on the host (unshard of
    the F-split) and scattered back to token positions.
  balance_loss is computed on device (launch 2) from the globally summed
  gating partials, so every output value is device-computed.
"""

import sys

if "/opt/trn_rl_repo" not in sys.path:
    sys.path.insert(0, "/opt/trn_rl_repo")

import numpy as np

import concourse.bass as bass
import concourse.mybir as mybir
import concourse.tile as tile
from concourse import bacc
from concourse.bass_utils import run_bass_kernel_spmd

F32 = mybir.dt.float32
F32R = mybir.dt.float32r
AF = mybir.ActivationFunctionType
ALU = mybir.AluOpType
AX = mybir.AxisListType

B, S, D = 8, 512, 768
E, F = 4, 3072
N = B * S                 # 4096 tokens
TPC = N // 8              # 512 tokens per core in the gating launch
DC = D // 128             # 6 contraction chunks of 128
FH = F // 2               # 1536 F-columns per core (F-split across the pair)
FT = FH // 128            # 12 f-tiles per core
C_DEFAULT = 1152          # per-expert token capacity (mean load is 1024)

_CACHE = {}


class _nullcm:
    def __enter__(self):
        return self

    def __exit__(self, *a):
        return False


def _chunks(c):
    """Split capacity C into matmul moving-dim chunks, each in [256, 512]
    (fp32r runs full-rate only for free dim >= 256). Per-span cost is
    max(LDW ~169ns, N/2.4 ns) per matmul group, so spans <= ~406 cost a
    flat LDW-bound price: pick the span count k minimizing total cost
    with a balanced split."""
    if c <= 512:
        parts = [c]
    else:
        best, parts = None, None
        kmin = -(-c // 512)
        for k in range(kmin, kmin + 3):
            # balanced split in multiples of 4 (fp32r ISA restriction)
            q = c // 4
            base, ext = divmod(q, k)
            cand = [4 * (base + 1)] * ext + [4 * base] * (k - ext)
            if min(cand) < 256 or max(cand) > 512:
                continue
            cost = sum(max(169.0, n / 2.4) for n in cand)
            if best is None or cost < best:
                best, parts = cost, cand
        assert parts is not None
    spans, t0 = [], 0
    for cn in parts:
        spans.append((t0, cn))
        t0 += cn
    assert t0 == c
    return spans


def _build_gate():
    nc = bacc.Bacc("TRN2", target_bir_lowering=False, debug=False, num_devices=8)
    xT = nc.dram_tensor("xT", [D, TPC], F32, kind="ExternalInput")
    gwT = nc.dram_tensor("gwT", [128, DC, E], F32, kind="ExternalInput")
    iota = nc.dram_tensor("iota", [128, E], F32, kind="ExternalInput")
    NT = TPC // 128
    ids = nc.dram_tensor("ids", [128, NT], F32, kind="ExternalOutput")
    pc = nc.dram_tensor("pc", [2 * NT * E, 1], F32, kind="ExternalOutput")

    xr = xT.ap().rearrange("(c p) t -> c p t", p=128)

    with tile.TileContext(nc) as tc:
        with (
            tc.tile_pool(name="sb", bufs=2) as sb,
            tc.tile_pool(name="ps", bufs=3, space="PSUM") as ps,
            tc.tile_pool(name="psT", bufs=2, space="PSUM") as psT,
            tc.tile_pool(name="psc", bufs=1, space="PSUM") as psc,
        ):
            gw = sb.tile([128, DC, E], F32, tag="gw")
            with tc.high_priority():
                nc.sync.dma_start(gw[:], gwT.ap())
            xs = [
                sb.tile([128, TPC], F32, tag=f"x{dcI}", name=f"xs{dcI}")
                for dcI in range(DC)
            ]
            # sub-tile DMAs so matmuls for token-tile tt start as soon as
            # its 128 columns are in (not the whole 512)
            for tt in range(NT):
                for dcI in range(DC):
                    nc.sync.dma_start(
                        xs[dcI][:, tt * 128:(tt + 1) * 128],
                        xr[dcI][:, tt * 128:(tt + 1) * 128],
                    )
            io = sb.tile([128, E], F32, tag="io")
            nc.sync.dma_start(io[:], iota[:])
            ones = sb.tile([128, 1], F32, tag="ones")
            nc.vector.memset(ones[:], 1.0)
            ident = sb.tile([E, E], F32, tag="ident")
            from concourse.masks import make_identity
            make_identity(nc, ident[:])

            # logits^T per 128-token tile with gw as the 4-column stationary
            # (LDWEIGHTS ~4 cols instead of a 128-col fp32 x-tile), then a
            # tiny PE transpose back to [128 tokens, E]
            lg = sb.tile([128, NT, E], F32, tag="lg")
            for tt in range(NT):
                plT = psT.tile([E, 128], F32, tag="plT")
                for dcI in range(DC):
                    nc.tensor.matmul(
                        plT[:],
                        gw[:, dcI, :],
                        xs[dcI][:, tt * 128:(tt + 1) * 128],
                        start=(dcI == 0),
                        stop=(dcI == DC - 1),
                    )
                lgT = sb.tile([E, 128], F32, tag="lgT")
                nc.vector.tensor_copy(lgT[:], plT[:])
                pl = ps.tile([128, E], F32, tag="pl")
                nc.tensor.transpose(pl[:], lgT[:], ident[:])
                nc.vector.tensor_copy(lg[:, tt, :], pl[:])

            m4 = sb.tile([128, NT], F32, tag="m4")
            nc.vector.tensor_reduce(m4[:], lg[:], AX.X, ALU.max)
            mb = m4[:, :, None].broadcast_to([128, NT, E])
            # big: [probs (NT,E) | mask (NT,E)]
            big = sb.tile([128, 2 * NT * E], F32, tag="big")
            bigv = big[:].rearrange("p (k t e) -> p k t e", k=2, t=NT)
            ex = sb.tile([128, NT, E], F32, tag="ex")
            nc.vector.tensor_tensor(ex[:], lg[:], mb, ALU.subtract)
            nc.scalar.activation(ex[:], ex[:], AF.Exp)
            s4 = sb.tile([128, NT], F32, tag="s4")
            nc.vector.tensor_reduce(s4[:], ex[:], AX.X, ALU.add)
            r4 = sb.tile([128, NT], F32, tag="r4")
            nc.vector.reciprocal(r4[:], s4[:])
            rb = r4[:, :, None].broadcast_to([128, NT, E])
            nc.vector.tensor_tensor(bigv[:, 0], ex[:], rb, ALU.mult)
            nc.vector.tensor_tensor(bigv[:, 1], lg[:], mb, ALU.is_ge)
            # ids = sum_e e * mask
            iob = io[:, None, :].broadcast_to([128, NT, E])
            tmp = sb.tile([128, NT, E], F32, tag="tmp")
            nc.vector.tensor_tensor(tmp[:], bigv[:, 1], iob, ALU.mult)
            ids_sb = sb.tile([128, NT], F32, tag="ids")
            nc.vector.tensor_reduce(ids_sb[:], tmp[:], AX.X, ALU.add)
            # column sums over the 128 tokens on partitions: ones matmul
            ppc = psc.tile([2 * NT * E, 1], F32, tag="ppc")
            nc.tensor.matmul(ppc[:], big[:], ones[:], start=True, stop=True)
            pc_sb = sb.tile([2 * NT * E, 1], F32, tag="pc")
            nc.vector.tensor_copy(pc_sb[:], ppc[:])
            nc.sync.dma_start(pc[:], pc_sb[:])
            nc.sync.dma_start(ids[:], ids_sb[:])
    nc.compile()
    return nc


def _build_expert(cap):
    # first span is a host-packed 256-column "first bite" (single DMA with
    # 6KB contiguous lines) so the first matmul group starts ~5us earlier
    if cap > 512:
        spans = [(0, 256)] + [(t0 + 256, tn) for t0, tn in _chunks(cap - 256)]
    else:
        spans = _chunks(cap)
    NS = len(spans)
    nc = bacc.Bacc("TRN2", target_bir_lowering=False, debug=False, num_devices=8)
    xT = nc.dram_tensor("xT", [D, cap], F32, kind="ExternalInput")
    x0 = nc.dram_tensor("x0", [128, DC, 256], F32, kind="ExternalInput")
    # w1 host layout: [FT, 128(p=f%128), DC, 128(d%128)] -> per-ft slab
    w1 = nc.dram_tensor("w1", [FT, 128, DC, 128], F32, kind="ExternalInput")
    b1 = nc.dram_tensor("b1", [128, FT], F32, kind="ExternalInput")
    w2 = nc.dram_tensor("w2", [FH, D], F32, kind="ExternalInput")
    b2 = nc.dram_tensor("b2", [128, DC], F32, kind="ExternalInput")
    pcin = nc.dram_tensor("pcin", [1, 2 * E], F32, kind="ExternalInput")
    outT = nc.dram_tensor("outT", [D, cap], F32, kind="ExternalOutput")
    bal = nc.dram_tensor("bal", [1, 1], F32, kind="ExternalOutput")

    xr = xT.ap().rearrange("(c p) t -> c p t", p=128)
    w2r_d = w2.ap().rearrange("(c p) d -> c p d", p=128)
    outr = outT.ap().rearrange("(c p) t -> c p t", p=128)

    with tile.TileContext(nc) as tc:
        with (
            tc.tile_pool(name="stage", bufs=3) as stage,
            tc.tile_pool(name="res", bufs=1) as res,
            tc.tile_pool(name="hp", bufs=1) as hp,
            tc.tile_pool(name="op", bufs=3) as op,
            tc.tile_pool(name="psA", bufs=4, space="PSUM") as psA,
            tc.tile_pool(name="psB", bufs=4, space="PSUM") as psB,
        ):
            b1_sb = res.tile([128, FT], F32, tag="b1")
            nc.sync.dma_start(b1_sb[:], b1.ap())
            b2_sb = res.tile([128, DC], F32, tag="b2")
            nc.sync.dma_start(b2_sb[:], b2.ap())

            # x: stream + round per (dc, span); w1 slabs interleaved so the
            # first matmuls (ft0, span0) have their inputs as early as
            # possible while later slabs stream during compute
            xrr = [[None] * NS for _ in range(DC)]
            w1t = [None] * FT

            def _load_w1(ft):
                st = stage.tile([128, DC, 128], F32, tag="w1s")
                nc.sync.dma_start(st[:], w1.ap()[ft])
                rt = res.tile([128, DC, 128], F32R, tag=f"w1r{ft}")
                nc.vector.tensor_copy(rt[:], st[:])
                w1t[ft] = rt

            # PE consumes: (ft0, span0), (ft0, span1), ... so emit x spans
            # first (all needed within the first ~10us) with one w1 slab
            # interleaved per span, then stream the remaining slabs
            fastbite = cap > 512
            if fastbite:
                x0st = stage.tile([128, DC, 256], F32, tag="x0s")
                nc.sync.dma_start(x0st[:], x0.ap())
                for dcI in range(DC):
                    rt = res.tile([128, 256], F32R, tag=f"xr{dcI}_0", name=f"x0r{dcI}")
                    nc.vector.tensor_copy(rt[:], x0st[:, dcI, :])
                    xrr[dcI][0] = rt
                _load_w1(0)
            for si, (t0, tn) in enumerate(spans):
                if fastbite and si == 0:
                    continue
                for dcI in range(DC):
                    st = stage.tile([128, tn], F32, tag="xs")
                    nc.sync.dma_start(st[:], xr[dcI][:, t0:t0 + tn])
                    rt = res.tile([128, tn], F32R, tag=f"xr{dcI}_{si}")
                    nc.vector.tensor_copy(rt[:], st[:])
                    xrr[dcI][si] = rt
                if si < FT:
                    _load_w1(si)
            for ft in range(FT):
                if w1t[ft] is None:
                    _load_w1(ft)

            hs = []
            for ft in range(FT):
                h = hp.tile([128, cap], F32R, tag=f"h{ft}")
                hs.append(h)

            # layer 1: h = gelu(W1h^T @ xT + b1h), written as fp32r by ACT
            for ft in range(FT):
                for si, (t0, tn) in enumerate(spans):
                    ph = psA.tile([128, tn], F32, tag="ph")
                    for dcI in range(DC):
                        nc.tensor.matmul(
                            ph[:],
                            w1t[ft][:, dcI, :],
                            xrr[dcI][si][:],
                            start=(dcI == 0),
                            stop=(dcI == DC - 1),
                        )
                    nc.scalar.activation(
                        hs[ft][:, t0:t0 + tn], ph[:], AF.Gelu,
                        bias=b1_sb[:, ft:ft + 1], scale=1.0,
                    )

            # w2 loads are emitted after L1 so they don't crowd the early
            # DMA/DVE critical path; they complete during L1 compute
            w2t = []
            for fc in range(FT):
                st = stage.tile([128, D], F32, tag="w2s")
                nc.sync.dma_start(st[:], w2r_d[fc])
                rt = res.tile([128, D], F32R, tag=f"w2r{fc}")
                nc.vector.tensor_copy(rt[:], st[:])
                w2t.append(rt)

            # layer 2: outT = W2h^T @ h + b2 (d-major, per-partition bias)
            for si, (t0, tn) in enumerate(spans):
                for db in range(DC):
                    po = psB.tile([128, tn], F32, tag="po")
                    for fc in range(FT):
                        nc.tensor.matmul(
                            po[:],
                            w2t[fc][:, db * 128:(db + 1) * 128],
                            hs[fc][:, t0:t0 + tn],
                            start=(fc == 0),
                            stop=(fc == FT - 1),
                        )
                    ot = op.tile([128, tn], F32, tag="ot")
                    nc.scalar.activation(
                        ot[:], po[:], AF.Identity, bias=b2_sb[:, db:db + 1], scale=1.0,
                    )
                    nc.sync.dma_start(outr[db][:, t0:t0 + tn], ot[:])

            # balance loss from global gating partials
            pc_sb = res.tile([1, 2 * E], F32, tag="pc")
            nc.sync.dma_start(pc_sb[:], pcin.ap())
            tmp4 = res.tile([1, E], F32, tag="tmp4")
            nc.vector.tensor_tensor(tmp4[:], pc_sb[0:1, 0:E], pc_sb[0:1, E:2 * E], ALU.mult)
            bsum = res.tile([1, 1], F32, tag="bsum")
            nc.vector.tensor_reduce(bsum[:], tmp4[:], AX.X, ALU.add)
            bal_sb = res.tile([1, 1], F32, tag="bal")
            nc.scalar.mul(bal_sb[:], bsum[:], float(E) / float(N) / float(N))
            nc.sync.dma_start(bal.ap(), bal_sb[:])
    nc.compile()
    return nc


def _get_gate():
    if "gate" not in _CACHE:
        _CACHE["gate"] = _build_gate()
    return _CACHE["gate"]


def _get_expert(cap):
    key = ("exp", cap)
    if key not in _CACHE:
        _CACHE[key] = _build_expert(cap)
    return _CACHE[key]


def _run(inputs, trace=False):
    x = np.ascontiguousarray(np.asarray(inputs["x"], dtype=np.float32))
    gate_w = np.asarray(inputs["gate_w"], dtype=np.float32)
    W1 = np.asarray(inputs["W1"], dtype=np.float32)
    b1 = np.asarray(inputs["b1"], dtype=np.float32)
    W2 = np.asarray(inputs["W2"], dtype=np.float32)
    b2 = np.asarray(inputs["b2"], dtype=np.float32)

    xf = x.reshape(N, D)
    xT = np.ascontiguousarray(xf.T)                      # [768, 4096]
    # [128(p=d%128), DC, E]: contiguous per-partition lines for DMA
    gwT = np.ascontiguousarray(gate_w.T.reshape(DC, 128, E).transpose(1, 0, 2))
    iota = np.broadcast_to(
        np.arange(E, dtype=np.float32), (128, E)
    ).copy()

    # ---- launch 1: gating ----
    nc_g = _get_gate()
    in_maps = []
    for c in range(8):
        in_maps.append({
            "xT": np.ascontiguousarray(xT[:, c * TPC:(c + 1) * TPC]),
            "gwT": gwT,
            "iota": iota,
        })
    res_g = run_bass_kernel_spmd(nc_g, in_maps, core_ids=list(range(8)), trace=trace)
    t_gate = res_g.exec_time_ns

    NT = TPC // 128
    gate = np.concatenate(
        [res_g.results[c]["ids"].T.reshape(-1) for c in range(8)]
    )
    gate = np.rint(gate).astype(np.int64)
    # pc rows: [k(probs/mask), tt, e]
    pcs = np.sum([res_g.results[c]["pc"][:, 0] for c in range(8)], axis=0)
    pcs = pcs.reshape(2, NT, E).sum(axis=1)
    probsum, counts = pcs[0], pcs[1]
    gate_load = np.rint(counts).astype(np.int32)

    # ---- host all-to-all by gate id ----
    idx = [np.flatnonzero(gate == e) for e in range(E)]
    max_load = max(len(i) for i in idx)
    # cap > ~1400 would exceed SBUF; extreme imbalance runs multiple batches
    CAP_MAX = 1408
    if max_load <= CAP_MAX:
        cap = max(512, ((max_load + 31) // 32) * 32)
        n_batches = 1
    else:
        cap = CAP_MAX
        n_batches = -(-max_load // CAP_MAX)
    nc_e = _get_expert(cap)

    pcin = np.concatenate([probsum, counts]).astype(np.float32)[None, :]
    zeros_b2 = np.zeros_like(b2[0])
    wmaps = []
    for core in range(8):
        e, h = core // 2, core % 2
        w1h = W1[e][:, h * FH:(h + 1) * FH]              # [768, 1536]
        # -> [FT, 128(d%128), DC, 128(f%128)]: lhsT needs d on partitions
        w1h = np.ascontiguousarray(
            w1h.reshape(DC, 128, FT, 128).transpose(2, 1, 0, 3)
        )
        b1h = np.ascontiguousarray(b1[e][h * FH:(h + 1) * FH].reshape(FT, 128).T)
        w2h = np.ascontiguousarray(W2[e][h * FH:(h + 1) * FH, :])
        b2h = b2[e] if h == 0 else zeros_b2
        b2h = np.ascontiguousarray(b2h.reshape(DC, 128).T)
        wmaps.append({"w1": w1h, "b1": b1h, "w2": w2h, "b2": b2h, "pcin": pcin})

    out_flat = np.empty((N, D), np.float32)
    t_exp = 0
    balance_loss = np.float32(0.0)
    for bi in range(n_batches):
        bidx = [i[bi * cap:(bi + 1) * cap] for i in idx]
        in_maps = []
        xsel = []
        x0sel = []
        for e in range(E):
            xs = np.zeros((D, cap), np.float32)
            xs[:, :len(bidx[e])] = xT[:, bidx[e]]
            xsel.append(xs)
            if cap > 512:
                x0 = np.ascontiguousarray(
                    xs[:, :256].reshape(DC, 128, 256).transpose(1, 0, 2)
                )
            else:
                x0 = np.zeros((128, DC, 256), np.float32)
            x0sel.append(x0)
        for core in range(8):
            in_maps.append(
                {"xT": xsel[core // 2], "x0": x0sel[core // 2], **wmaps[core]}
            )
        res_e = run_bass_kernel_spmd(
            nc_e, in_maps, core_ids=list(range(8)), trace=trace
        )
        if res_e.exec_time_ns is not None:
            t_exp += res_e.exec_time_ns
        for e in range(E):
            if len(bidx[e]) == 0:
                continue
            oT = res_e.results[2 * e]["outT"] + res_e.results[2 * e + 1]["outT"]
            out_flat[bidx[e]] = oT[:, :len(bidx[e])].T
        balance_loss = np.float32(res_e.results[0]["bal"][0, 0])
    out = out_flat.reshape(B, S, D)
    if trace and t_exp == 0:
        t_exp = None

    times = (t_gate, t_exp)
    return (out, balance_loss, gate_load), times


def kernel(**inputs):
    (out, balance_loss, gate_load), _ = _run(inputs, trace=False)
    return out, balance_loss, gate_load
